# revision 23
# baseline (speedup 1.0000x reference)
"""AdapterAttention Trainium2 kernel (8 NeuronCores, batch-data-parallel).

Reference computation (per image, N=197 tokens, C=768, 12 heads x 64):
    mid       = tanh(x @ down_w.T + down_b)                  [N, 256]
    kv_prefix = mid @ up_w.T + up_b                          [N, 1536]
    qkv       = x @ qkv_w.T + qkv_b + 0.8*[0, kv_prefix]     [N, 2304]
    attn      = softmax(q k^T / 8) v  per head
    out       = attn_out @ proj_w.T + proj_b

Distribution: batch 64 -> 8 images per core, weights replicated. No
collectives; host shards inputs and reassembles outputs.

On-chip layout strategy (everything "feature-major"/transposed so no
on-chip transposes are needed; host pre-transposes x and weights):
    xT    [c=768, t=1576]  (t = 8 images * 197 tokens)
    qT/kT [o=768, t]  = W^T-matmuls on xT (+ adapter matmuls for kT)
    V_aug per image, natural [tok chunks, 12*(64+1)]; the extra column
          per head is constant 1.0, so the AV matmul's PSUM row 64
          yields the softmax denominator for free.
    S^T   [j, i] per (image, head) = k-slice^T-matmul(q) -> one exp op
          (bf16 out IS the E^T eviction, no separate copy)
    O^T   [65, i] = V_aug-lhsT matmul(E^T); normalization = reciprocal
          of row 64, gpsimd partition-broadcast, fused into the output
          eviction as a tensor_tensor multiply.
    proj  -> outT [co, t]; host transposes back.
The v-bias is folded into proj bias on the host (softmax rows sum to 1);
0.8 prefix scale is folded into up_w/up_b on the host.
"""

import sys

sys.path.insert(0, "/opt/trn_rl_repo")

import ml_dtypes
import numpy as np

DIM = 768
MID = 256
HEADS = 12
D = 64
P_SCALE = 0.8
SCALE = D ** -0.5
B_FULL = 64
N_TOK = 197
N_CORES = 8
B_LOC = B_FULL // N_CORES          # 8 images per core
T = B_LOC * N_TOK                  # 1576 tokens per core
NT = 394                           # t-chunk for dense matmuls (= 2 images)
N_NT = T // NT                     # 4
CC = DIM // 128                    # 6 contraction chunks over c
MC = MID // 128                    # 2 contraction chunks over mid
OT = DIM // 128                    # 6 output tiles over o / co
HW = D + 1                         # 65: per-head V columns incl. ones col

BF16 = ml_dtypes.bfloat16

_CACHE = {}
SAFE_EXP = False


def _build():
    from contextlib import ExitStack

    import concourse.tile as tile
    from concourse import bacc, mybir

    dt = mybir.dt
    AF = mybir.ActivationFunctionType

    nc = bacc.Bacc(
        "TRN2", target_bir_lowering=False, debug=False, num_devices=N_CORES
    )

    def din(name, shape, dtype):
        return nc.dram_tensor(name, shape, dtype, kind="ExternalInput").ap()

    xT_d = din("xT", [DIM, T], dt.bfloat16)
    wq_d = din("wq", [DIM, DIM], dt.bfloat16)      # [c, o]
    wk_d = din("wk", [DIM, DIM], dt.bfloat16)
    wv_d = din("wv", [DIM, DIM], dt.bfloat16)
    dw_d = din("dw", [DIM, MID], dt.bfloat16)      # down_w.T  [c, m]
    uk_d = din("uk", [MID, DIM], dt.bfloat16)      # 0.8*up_w[:C].T  [m, o]
    uv_d = din("uv", [MID, DIM], dt.bfloat16)      # 0.8*up_w[C:].T  [m, o]
    pw_d = din("pw", [DIM, DIM], dt.bfloat16)      # proj_w.T  [o, co]
    bq_d = din("bq", [128, OT], dt.float32)
    bk_d = din("bk", [128, OT], dt.float32)
    bd_d = din("bd", [128, MC], dt.float32)
    bp_d = din("bp", [128, OT], dt.float32)
    out_d = nc.dram_tensor("outT", [DIM, T], dt.float32, kind="ExternalOutput").ap()

    with tile.TileContext(nc) as tc, ExitStack() as ctx:
        persist = ctx.enter_context(tc.tile_pool(name="persist", bufs=1))

        xT = persist.tile([128, CC, T], dt.bfloat16)
        xT_v = xT_d.rearrange("(c p) t -> p c t", p=128)
        nc.sync.dma_start(out=xT[:, :, 0:NT], in_=xT_v[:, :, 0:NT])
        dw = persist.tile([128, CC, MID], dt.bfloat16)
        nc.sync.dma_start(out=dw[:], in_=dw_d.rearrange("(c p) m -> p c m", p=128))
        wq = persist.tile([128, CC, DIM], dt.bfloat16)
        nc.sync.dma_start(out=wq[:], in_=wq_d.rearrange("(c p) o -> p c o", p=128))
        # later-needed weights are DMA'd from the Scalar engine's stream at
        # points inside stage A(nt=0), so their transfers don't steal HBM
        # bandwidth from dw/xT/wq which gate the first matmuls
        wk = persist.tile([128, CC, DIM], dt.bfloat16)
        uk = persist.tile([128, MC, DIM], dt.bfloat16)
        wv = persist.tile([128, CC, DIM], dt.bfloat16)
        uv = persist.tile([128, MC, DIM], dt.bfloat16)
        pw = persist.tile([128, OT, DIM], dt.bfloat16)
        bq = persist.tile([128, OT], dt.float32)
        nc.sync.dma_start(out=bq[:], in_=bq_d)
        bk = persist.tile([128, OT], dt.float32)
        nc.sync.dma_start(out=bk[:], in_=bk_d)
        bd = persist.tile([128, MC], dt.float32)
        nc.sync.dma_start(out=bd[:], in_=bd_d)
        bp = persist.tile([128, OT], dt.float32)
        nc.sync.dma_start(out=bp[:], in_=bp_d)
        for _nt in range(1, N_NT):
            _sl = slice(_nt * NT, (_nt + 1) * NT)
            nc.sync.dma_start(out=xT[:, :, _sl], in_=xT_v[:, :, _sl])

        # per-nt-chunk activation tiles so attention can overlap stage A
        qT = [persist.tile([128, OT, NT], dt.bfloat16, name=f"qT{i}") for i in range(N_NT)]
        kT = [persist.tile([128, OT, NT], dt.bfloat16, name=f"kT{i}") for i in range(N_NT)]
        OTt = [persist.tile([128, OT, NT], dt.bfloat16, name=f"OTt{i}") for i in range(N_NT)]
        midT = [persist.tile([128, MC, NT], dt.bfloat16, name=f"midT{i}") for i in range(N_NT)]
        # V augmented with a ones column per head: [tok, 12*(64+1)]
        V = [
            persist.tile([128, 2, HEADS * HW], dt.bfloat16, name=f"V{b}")
            for b in range(B_LOC)
        ]
        for b in range(B_LOC):
            ones_cols = V[b][:].rearrange("p j (h c) -> p j h c", h=HEADS)[
                :, :, :, D:HW
            ]
            nc.vector.memset(ones_cols, 1.0)

        uPool = ctx.enter_context(tc.tile_pool(name="unorm", bufs=2))
        U = [
            uPool.tile([HW, N_TOK], dt.float32, tag=f"u{i}", name=f"u{i}")
            for i in range(2 * HEADS)
        ]
        with (
            tc.tile_pool(name="psAB", bufs=2, space="PSUM") as pA,
            tc.tile_pool(name="psS", bufs=2, space="PSUM") as pS,
            tc.tile_pool(name="psO", bufs=2, space="PSUM") as pO,
            tc.tile_pool(name="psD", bufs=2, space="PSUM") as pD,
            tc.tile_pool(name="att", bufs=3) as pE,
            tc.tile_pool(name="rec", bufs=4) as pR,
        ):
            # ---- Stage A/B per t-chunk: midT, qT, kT, V ---------------
            for nt in range(N_NT):
                sl = slice(nt * NT, (nt + 1) * NT)
                for mt in range(MC):
                    ps0 = pA.tile([128, 512], dt.float32, tag="ps2k", name="psm")
                    ps = ps0[:, :NT]
                    for cc in range(CC):
                        nc.tensor.matmul(
                            ps[:],
                            lhsT=dw[:, cc, mt * 128:(mt + 1) * 128],
                            rhs=xT[:, cc, sl],
                            start=(cc == 0),
                            stop=(cc == CC - 1),
                        )
                    nc.scalar.activation(
                        out=midT[nt][:, mt, :], in_=ps[:], func=AF.Tanh,
                        bias=bd[:, mt:mt + 1], scale=1.0,
                    )
                if nt == 0:
                    nc.scalar.dma_start(
                        out=wk[:], in_=wk_d.rearrange("(c p) o -> p c o", p=128)
                    )
                    nc.scalar.dma_start(
                        out=uk[:], in_=uk_d.rearrange("(c p) o -> p c o", p=128)
                    )
                for ot in range(OT):
                    ps0 = pA.tile([128, 512], dt.float32, tag="ps2k", name="psq")
                    ps = ps0[:, :NT]
                    for cc in range(CC):
                        nc.tensor.matmul(
                            ps[:],
                            lhsT=wq[:, cc, ot * 128:(ot + 1) * 128],
                            rhs=xT[:, cc, sl],
                            start=(cc == 0),
                            stop=(cc == CC - 1),
                        )
                    nc.scalar.activation(
                        out=qT[nt][:, ot, :], in_=ps[:], func=AF.Identity,
                        bias=bq[:, ot:ot + 1], scale=1.0,
                    )
                    if nt == 0 and ot == 1:
                        nc.scalar.dma_start(
                            out=wv[:],
                            in_=wv_d.rearrange("(c p) o -> p c o", p=128),
                        )
                        nc.scalar.dma_start(
                            out=uv[:],
                            in_=uv_d.rearrange("(c p) o -> p c o", p=128),
                        )
                    if nt == 0 and ot == 3:
                        nc.scalar.dma_start(
                            out=pw[:],
                            in_=pw_d.rearrange("(c p) o -> p c o", p=128),
                        )
                for ot in range(OT):
                    ps0 = pA.tile([128, 512], dt.float32, tag="ps2k", name="psk")
                    ps = ps0[:, :NT]
                    for cc in range(CC):
                        nc.tensor.matmul(
                            ps[:],
                            lhsT=wk[:, cc, ot * 128:(ot + 1) * 128],
                            rhs=xT[:, cc, sl],
                            start=(cc == 0),
                            stop=False,
                        )
                    for mc in range(MC):
                        nc.tensor.matmul(
                            ps[:],
                            lhsT=uk[:, mc, ot * 128:(ot + 1) * 128],
                            rhs=midT[nt][:, mc, :],
                            start=False,
                            stop=(mc == MC - 1),
                        )
                    nc.scalar.activation(
                        out=kT[nt][:, ot, :], in_=ps[:], func=AF.Identity,
                        bias=bk[:, ot:ot + 1], scale=1.0,
                    )
                # V for the two images inside this t-chunk
                for b in (2 * nt, 2 * nt + 1):
                    t0 = b * N_TOK
                    l0 = (b % 2) * N_TOK     # image offset inside nt tiles
                    for jc in range(2):
                        jsz = 128 if jc == 0 else N_TOK - 128
                        tok = slice(t0 + jc * 128, t0 + jc * 128 + jsz)
                        ltok = slice(l0 + jc * 128, l0 + jc * 128 + jsz)
                        for osl, nh, h0 in (
                            (slice(0, 512), 8, 0),
                            (slice(512, DIM), 4, 8),
                        ):
                            ow = osl.stop - osl.start
                            ps = pA.tile([128, 512], dt.float32, tag="ps2k", name="psv")
                            for cc in range(CC):
                                nc.tensor.matmul(
                                    ps[:jsz, :ow],
                                    lhsT=xT[:, cc, tok],
                                    rhs=wv[:, cc, osl],
                                    start=(cc == 0),
                                    stop=False,
                                )
                            for mc in range(MC):
                                nc.tensor.matmul(
                                    ps[:jsz, :ow],
                                    lhsT=midT[nt][:, mc, ltok],
                                    rhs=uv[:, mc, osl],
                                    start=False,
                                    stop=(mc == MC - 1),
                                )
                            dst = V[b][:].rearrange(
                                "p j (h c) -> p j h c", h=HEADS
                            )[:jsz, jc, h0:h0 + nh, 0:D]
                            src = ps[:jsz, :ow].rearrange(
                                "p (h c) -> p h c", h=nh
                            )
                            if b % 2 == 0:
                                nc.vector.tensor_copy(out=dst, in_=src)
                            else:
                                nc.scalar.copy(out=dst, in_=src)

                # ---- attention for the two images of this chunk -------
                for b in (2 * nt, 2 * nt + 1):
                    i0 = (b % 2) * N_TOK     # image offset inside nt tiles
                    for hp in range(HEADS // 2):
                        oh = hp
                        # the two heads of a pair sit at partition bases 0
                        # and 64 of one kT/qT chunk: issuing their K=64
                        # matmuls back-to-back lets the PE run them
                        # concurrently in disjoint row-groups
                        sps = [
                            pS.tile([128, 2, N_TOK], dt.float32, tag="s",
                                    name=f"s{parity}")
                            for parity in range(2)
                        ]
                        for jc in range(2):
                            jsz = 128 if jc == 0 else N_TOK - 128
                            jsl = slice(i0 + jc * 128, i0 + jc * 128 + jsz)
                            for parity in range(2):
                                po = 64 * parity
                                nc.tensor.matmul(
                                    sps[parity][:jsz, jc, :],
                                    lhsT=kT[nt][po:po + 64, oh, jsl],
                                    rhs=qT[nt][po:po + 64, oh, i0:i0 + N_TOK],
                                    start=True,
                                    stop=True,
                                )
                        for parity in range(2):
                            h = 2 * hp + parity
                            s_ps = sps[parity]
                            eT = pE.tile([128, 2, N_TOK], dt.bfloat16, tag="eT")
                            # one exp over both j-chunks; rows 69.. of chunk
                            # 1 are stale psum, never read downstream
                            nc.scalar.activation(
                                out=eT[:], in_=s_ps[:], func=AF.Exp, scale=SCALE
                            )
                            o_ps = pO.tile([HW, N_TOK], dt.float32, tag="o")
                            for jc in range(2):
                                jsz = 128 if jc == 0 else N_TOK - 128
                                nc.tensor.matmul(
                                    o_ps[:],
                                    lhsT=V[b][:jsz, jc, h * HW:(h + 1) * HW],
                                    rhs=eT[:jsz, jc, :],
                                    start=(jc == 0),
                                    stop=(jc == 1),
                                )
                            # fast evict (frees the PSUM slot); normalization
                            # is a deferred batched pass below
                            u = U[(b % 2) * HEADS + h]
                            if parity == 0:
                                nc.scalar.copy(out=u[:], in_=o_ps[:])
                            else:
                                nc.vector.tensor_copy(out=u[:], in_=o_ps[:])

                # ---- deferred normalization for this chunk ------------
                for b in (2 * nt, 2 * nt + 1):
                    i0 = (b % 2) * N_TOK
                    for h in range(HEADS):
                        po = 64 * (h % 2)
                        oh = h // 2
                        u = U[(b % 2) * HEADS + h]
                        r0 = pR.tile([1, N_TOK], dt.float32, tag="r0")
                        nc.vector.tensor_copy(out=r0[:], in_=u[D:HW, :])
                        r1 = pR.tile([1, N_TOK], dt.float32, tag="r1")
                        nc.vector.reciprocal_approx_fast(out=r1[:], in_=r0[:])
                        rb = pR.tile([64, N_TOK], dt.float32, tag="rb")
                        nc.gpsimd.partition_broadcast(rb[:], r1[:])
                        nc.vector.tensor_mul(
                            OTt[nt][po:po + 64, oh, i0:i0 + N_TOK],
                            u[0:D, :],
                            rb[:],
                        )

                # ---- projection for the previous t-chunk --------------
                for pnt in ([nt - 1] if nt > 0 else []) + (
                    [nt] if nt == N_NT - 1 else []
                ):
                    psl = slice(pnt * NT, (pnt + 1) * NT)
                    for ct in range(OT):
                        ps = pD.tile([128, NT], dt.float32, tag="psd", name="psd")
                        for oc in range(OT):
                            nc.tensor.matmul(
                                ps[:],
                                lhsT=pw[:, oc, ct * 128:(ct + 1) * 128],
                                rhs=OTt[pnt][:, oc, :],
                                start=(oc == 0),
                                stop=(oc == OT - 1),
                            )
                        st = pE.tile([128, NT], dt.float32, tag="st")
                        nc.scalar.activation(
                            out=st[:], in_=ps[:], func=AF.Identity,
                            bias=bp[:, ct:ct + 1], scale=1.0,
                        )
                        nc.sync.dma_start(
                            out=out_d[ct * 128:(ct + 1) * 128, psl], in_=st[:]
                        )

    nc.compile()
    return nc


def _prep_inputs(x, qkv_w, qkv_b, proj_w, proj_b, down_w, down_b, up_w, up_b):
    f32 = np.float32
    x = np.asarray(x, f32)
    qkv_w = np.asarray(qkv_w, f32)
    qkv_b = np.asarray(qkv_b, f32)
    proj_w = np.asarray(proj_w, f32)
    proj_b = np.asarray(proj_b, f32)
    down_w = np.asarray(down_w, f32)
    down_b = np.asarray(down_b, f32)
    up_w = np.asarray(up_w, f32)
    up_b = np.asarray(up_b, f32)

    wq = qkv_w[0:DIM]
    wk = qkv_w[DIM:2 * DIM]
    wv = qkv_w[2 * DIM:3 * DIM]
    bq = qkv_b[0:DIM]
    bk = qkv_b[DIM:2 * DIM] + P_SCALE * up_b[0:DIM]
    bv = qkv_b[2 * DIM:3 * DIM] + P_SCALE * up_b[DIM:2 * DIM]
    # v-bias rides through the softmax average unchanged -> fold into proj_b
    bp = proj_b + proj_w @ bv

    def t_bf16(a):
        return np.ascontiguousarray(a.T).astype(BF16)

    def b_lay(vec, nt):
        return np.ascontiguousarray(vec.reshape(nt, 128).T).astype(f32)

    common = {
        "wq": t_bf16(wq),
        "wk": t_bf16(wk),
        "wv": t_bf16(wv),
        "dw": t_bf16(down_w),
        "uk": t_bf16(P_SCALE * up_w[0:DIM]),
        "uv": t_bf16(P_SCALE * up_w[DIM:2 * DIM]),
        "pw": t_bf16(proj_w),
        "bq": b_lay(bq, OT),
        "bk": b_lay(bk, OT),
        "bd": b_lay(down_b, MC),
        "bp": b_lay(bp, OT),
    }
    in_maps = []
    for c in range(N_CORES):
        xc = x[c * B_LOC:(c + 1) * B_LOC].reshape(T, DIM)
        m = dict(common)
        m["xT"] = np.ascontiguousarray(xc.T).astype(BF16)
        in_maps.append(m)
    return in_maps


def kernel(x, qkv_w, qkv_b, proj_w, proj_b, down_w, down_b, up_w, up_b):
    from concourse.bass_utils import run_bass_kernel_spmd

    if "nc" not in _CACHE:
        _CACHE["nc"] = _build()
    nc = _CACHE["nc"]

    in_maps = _prep_inputs(
        x, qkv_w, qkv_b, proj_w, proj_b, down_w, down_b, up_w, up_b
    )
    res = run_bass_kernel_spmd(nc, in_maps, list(range(N_CORES)))
    outs = []
    for i in range(N_CORES):
        oT = np.asarray(res.results[i]["outT"], dtype=np.float32)
        outs.append(np.ascontiguousarray(oT.T).reshape(B_LOC, N_TOK, DIM))
    return np.concatenate(outs, axis=0)


# revision 24
# speedup vs baseline: 1.0480x; 1.0480x over previous
"""AdapterAttention Trainium2 kernel (8 NeuronCores, batch-data-parallel).

Reference computation (per image, N=197 tokens, C=768, 12 heads x 64):
    mid       = tanh(x @ down_w.T + down_b)                  [N, 256]
    kv_prefix = mid @ up_w.T + up_b                          [N, 1536]
    qkv       = x @ qkv_w.T + qkv_b + 0.8*[0, kv_prefix]     [N, 2304]
    attn      = softmax(q k^T / 8) v  per head
    out       = attn_out @ proj_w.T + proj_b

Distribution: batch 64 -> 8 images per core, weights replicated. No
collectives; host shards inputs and reassembles outputs.

On-chip layout strategy (everything "feature-major"/transposed so no
on-chip transposes are needed; host pre-transposes x and weights):
    xT    [c=768, t=1576]  (t = 8 images * 197 tokens)
    qT/kT [o=768, t]  = W^T-matmuls on xT (+ adapter matmuls for kT)
    V_aug per image, natural [tok chunks, 12*(64+1)]; the extra column
          per head is constant 1.0, so the AV matmul's PSUM row 64
          yields the softmax denominator for free.
    S^T   [j, i] per (image, head) = k-slice^T-matmul(q) -> one exp op
          (bf16 out IS the E^T eviction, no separate copy)
    O^T   [65, i] = V_aug-lhsT matmul(E^T); normalization = reciprocal
          of row 64, gpsimd partition-broadcast, fused into the output
          eviction as a tensor_tensor multiply.
    proj  -> outT [co, t]; host transposes back.
The v-bias is folded into proj bias on the host (softmax rows sum to 1);
0.8 prefix scale is folded into up_w/up_b on the host.
"""

import sys

sys.path.insert(0, "/opt/trn_rl_repo")

import ml_dtypes
import numpy as np

DIM = 768
MID = 256
HEADS = 12
D = 64
P_SCALE = 0.8
SCALE = D ** -0.5
B_FULL = 64
N_TOK = 197
N_CORES = 8
B_LOC = B_FULL // N_CORES          # 8 images per core
T = B_LOC * N_TOK                  # 1576 tokens per core
NT = 394                           # t-chunk for dense matmuls (= 2 images)
N_NT = T // NT                     # 4
CC = DIM // 128                    # 6 contraction chunks over c
MC = MID // 128                    # 2 contraction chunks over mid
OT = DIM // 128                    # 6 output tiles over o / co
HW = D + 1                         # 65: per-head V columns incl. ones col

BF16 = ml_dtypes.bfloat16

_CACHE = {}
SAFE_EXP = False


def _build():
    from contextlib import ExitStack

    import concourse.tile as tile
    from concourse import bacc, mybir

    dt = mybir.dt
    AF = mybir.ActivationFunctionType

    nc = bacc.Bacc(
        "TRN2", target_bir_lowering=False, debug=False, num_devices=N_CORES
    )

    def din(name, shape, dtype):
        return nc.dram_tensor(name, shape, dtype, kind="ExternalInput").ap()

    xT_d = din("xT", [DIM, T], dt.bfloat16)
    wq_d = din("wq", [DIM, DIM], dt.bfloat16)      # [c, o]
    wk_d = din("wk", [DIM, DIM], dt.bfloat16)
    wv_d = din("wv", [DIM, DIM], dt.bfloat16)
    dw_d = din("dw", [DIM, MID], dt.bfloat16)      # down_w.T  [c, m]
    uk_d = din("uk", [MID, DIM], dt.bfloat16)      # 0.8*up_w[:C].T  [m, o]
    uv_d = din("uv", [MID, DIM], dt.bfloat16)      # 0.8*up_w[C:].T  [m, o]
    pw_d = din("pw", [DIM, DIM], dt.bfloat16)      # proj_w.T  [o, co]
    bq_d = din("bq", [128, OT], dt.float32)
    bk_d = din("bk", [128, OT], dt.float32)
    bd_d = din("bd", [128, MC], dt.float32)
    bp_d = din("bp", [128, OT], dt.float32)
    out_d = nc.dram_tensor("outT", [DIM, T], dt.float32, kind="ExternalOutput").ap()

    with tile.TileContext(nc) as tc, ExitStack() as ctx:
        persist = ctx.enter_context(tc.tile_pool(name="persist", bufs=1))

        xT = persist.tile([128, CC, T], dt.bfloat16)
        xT_v = xT_d.rearrange("(c p) t -> p c t", p=128)
        nc.sync.dma_start(out=xT[:, :, 0:NT], in_=xT_v[:, :, 0:NT])
        dw = persist.tile([128, CC, MID], dt.bfloat16)
        nc.sync.dma_start(out=dw[:], in_=dw_d.rearrange("(c p) m -> p c m", p=128))
        wq = persist.tile([128, CC, DIM], dt.bfloat16)
        nc.sync.dma_start(out=wq[:], in_=wq_d.rearrange("(c p) o -> p c o", p=128))
        # later-needed weights are DMA'd from the Scalar engine's stream at
        # points inside stage A(nt=0), so their transfers don't steal HBM
        # bandwidth from dw/xT/wq which gate the first matmuls
        wk = persist.tile([128, CC, DIM], dt.bfloat16)
        uk = persist.tile([128, MC, DIM], dt.bfloat16)
        wv = persist.tile([128, CC, DIM], dt.bfloat16)
        uv = persist.tile([128, MC, DIM], dt.bfloat16)
        pw = persist.tile([128, OT, DIM], dt.bfloat16)
        bq = persist.tile([128, OT], dt.float32)
        nc.sync.dma_start(out=bq[:], in_=bq_d)
        bk = persist.tile([128, OT], dt.float32)
        nc.sync.dma_start(out=bk[:], in_=bk_d)
        bd = persist.tile([128, MC], dt.float32)
        nc.sync.dma_start(out=bd[:], in_=bd_d)
        bp = persist.tile([128, OT], dt.float32)
        nc.sync.dma_start(out=bp[:], in_=bp_d)
        for _nt in range(1, N_NT):
            _sl = slice(_nt * NT, (_nt + 1) * NT)
            nc.sync.dma_start(out=xT[:, :, _sl], in_=xT_v[:, :, _sl])

        # per-nt-chunk activation tiles so attention can overlap stage A
        qT = [persist.tile([128, OT, NT], dt.bfloat16, name=f"qT{i}") for i in range(N_NT)]
        kT = [persist.tile([128, OT, NT], dt.bfloat16, name=f"kT{i}") for i in range(N_NT)]
        OTt = [persist.tile([128, OT, NT], dt.bfloat16, name=f"OTt{i}") for i in range(N_NT)]
        midT = [persist.tile([128, MC, NT], dt.bfloat16, name=f"midT{i}") for i in range(N_NT)]
        # V augmented with a ones column per head: [tok, 12*(64+1)]
        V = [
            persist.tile([128, 2, HEADS * HW], dt.bfloat16, name=f"V{b}")
            for b in range(B_LOC)
        ]
        for b in range(B_LOC):
            ones_cols = V[b][:].rearrange("p j (h c) -> p j h c", h=HEADS)[
                :, :, :, D:HW
            ]
            nc.vector.memset(ones_cols, 1.0)

        uPool = ctx.enter_context(tc.tile_pool(name="unorm", bufs=2))
        U = [
            uPool.tile([HW, N_TOK], dt.float32, tag=f"u{i}", name=f"u{i}")
            for i in range(2 * HEADS)
        ]
        with (
            tc.tile_pool(name="psAB", bufs=2, space="PSUM") as pA,
            tc.tile_pool(name="psS", bufs=2, space="PSUM") as pS,
            tc.tile_pool(name="psO", bufs=2, space="PSUM") as pO,
            tc.tile_pool(name="att", bufs=3) as pE,
            tc.tile_pool(name="rec", bufs=4) as pR,
        ):
            # ---- Stage A/B per t-chunk: midT, qT, kT, V ---------------
            for nt in range(N_NT):
                sl = slice(nt * NT, (nt + 1) * NT)
                for mt in range(MC):
                    ps0 = pA.tile([128, 512], dt.float32, tag="ps2k", name="psm")
                    ps = ps0[:, :NT]
                    for cc in range(CC):
                        nc.tensor.matmul(
                            ps[:],
                            lhsT=dw[:, cc, mt * 128:(mt + 1) * 128],
                            rhs=xT[:, cc, sl],
                            start=(cc == 0),
                            stop=(cc == CC - 1),
                        )
                    nc.scalar.activation(
                        out=midT[nt][:, mt, :], in_=ps[:], func=AF.Tanh,
                        bias=bd[:, mt:mt + 1], scale=1.0,
                    )
                if nt == 0:
                    nc.scalar.dma_start(
                        out=wk[:], in_=wk_d.rearrange("(c p) o -> p c o", p=128)
                    )
                    nc.scalar.dma_start(
                        out=uk[:], in_=uk_d.rearrange("(c p) o -> p c o", p=128)
                    )
                for ot in range(OT):
                    ps0 = pA.tile([128, 512], dt.float32, tag="ps2k", name="psq")
                    ps = ps0[:, :NT]
                    for cc in range(CC):
                        nc.tensor.matmul(
                            ps[:],
                            lhsT=wq[:, cc, ot * 128:(ot + 1) * 128],
                            rhs=xT[:, cc, sl],
                            start=(cc == 0),
                            stop=(cc == CC - 1),
                        )
                    nc.scalar.activation(
                        out=qT[nt][:, ot, :], in_=ps[:], func=AF.Identity,
                        bias=bq[:, ot:ot + 1], scale=1.0,
                    )
                    if nt == 0 and ot == 1:
                        nc.scalar.dma_start(
                            out=wv[:],
                            in_=wv_d.rearrange("(c p) o -> p c o", p=128),
                        )
                        nc.scalar.dma_start(
                            out=uv[:],
                            in_=uv_d.rearrange("(c p) o -> p c o", p=128),
                        )
                    if nt == 0 and ot == 3:
                        nc.scalar.dma_start(
                            out=pw[:],
                            in_=pw_d.rearrange("(c p) o -> p c o", p=128),
                        )
                for ot in range(OT):
                    ps0 = pA.tile([128, 512], dt.float32, tag="ps2k", name="psk")
                    ps = ps0[:, :NT]
                    for cc in range(CC):
                        nc.tensor.matmul(
                            ps[:],
                            lhsT=wk[:, cc, ot * 128:(ot + 1) * 128],
                            rhs=xT[:, cc, sl],
                            start=(cc == 0),
                            stop=False,
                        )
                    for mc in range(MC):
                        nc.tensor.matmul(
                            ps[:],
                            lhsT=uk[:, mc, ot * 128:(ot + 1) * 128],
                            rhs=midT[nt][:, mc, :],
                            start=False,
                            stop=(mc == MC - 1),
                        )
                    nc.scalar.activation(
                        out=kT[nt][:, ot, :], in_=ps[:], func=AF.Identity,
                        bias=bk[:, ot:ot + 1], scale=1.0,
                    )
                # V for the two images inside this t-chunk
                for b in (2 * nt, 2 * nt + 1):
                    t0 = b * N_TOK
                    l0 = (b % 2) * N_TOK     # image offset inside nt tiles
                    for jc in range(2):
                        jsz = 128 if jc == 0 else N_TOK - 128
                        tok = slice(t0 + jc * 128, t0 + jc * 128 + jsz)
                        ltok = slice(l0 + jc * 128, l0 + jc * 128 + jsz)
                        for osl, nh, h0 in (
                            (slice(0, 512), 8, 0),
                            (slice(512, DIM), 4, 8),
                        ):
                            ow = osl.stop - osl.start
                            ps = pA.tile([128, 512], dt.float32, tag="ps2k", name="psv")
                            for cc in range(CC):
                                nc.tensor.matmul(
                                    ps[:jsz, :ow],
                                    lhsT=xT[:, cc, tok],
                                    rhs=wv[:, cc, osl],
                                    start=(cc == 0),
                                    stop=False,
                                )
                            for mc in range(MC):
                                nc.tensor.matmul(
                                    ps[:jsz, :ow],
                                    lhsT=midT[nt][:, mc, ltok],
                                    rhs=uv[:, mc, osl],
                                    start=False,
                                    stop=(mc == MC - 1),
                                )
                            dst = V[b][:].rearrange(
                                "p j (h c) -> p j h c", h=HEADS
                            )[:jsz, jc, h0:h0 + nh, 0:D]
                            src = ps[:jsz, :ow].rearrange(
                                "p (h c) -> p h c", h=nh
                            )
                            if b % 2 == 0:
                                nc.vector.tensor_copy(out=dst, in_=src)
                            else:
                                nc.scalar.copy(out=dst, in_=src)

                # ---- attention for the two images of this chunk -------
                for b in (2 * nt, 2 * nt + 1):
                    i0 = (b % 2) * N_TOK     # image offset inside nt tiles
                    for hp in range(HEADS // 2):
                        oh = hp
                        # the two heads of a pair sit at partition bases 0
                        # and 64 of one kT/qT chunk: issuing their K=64
                        # matmuls back-to-back lets the PE run them
                        # concurrently in disjoint row-groups. One padded
                        # 2-bank tile holds both heads' scores.
                        spair = pS.tile(
                            [128, 2, 2, 256], dt.float32, tag="s", name="spair"
                        )
                        for jc in range(2):
                            jsz = 128 if jc == 0 else N_TOK - 128
                            jsl = slice(i0 + jc * 128, i0 + jc * 128 + jsz)
                            for parity in range(2):
                                po = 64 * parity
                                nc.tensor.matmul(
                                    spair[:jsz, parity, jc, 0:N_TOK],
                                    lhsT=kT[nt][po:po + 64, oh, jsl],
                                    rhs=qT[nt][po:po + 64, oh, i0:i0 + N_TOK],
                                    start=True,
                                    stop=True,
                                )
                        for parity in range(2):
                            h = 2 * hp + parity
                            eT = pE.tile([128, 2, N_TOK], dt.bfloat16, tag="eT")
                            # one exp over both j-chunks; rows 69.. of chunk
                            # 1 are stale psum, never read downstream
                            nc.scalar.activation(
                                out=eT[:],
                                in_=spair[:, parity, :, 0:N_TOK],
                                func=AF.Exp, scale=SCALE,
                            )
                            o_ps = pO.tile([HW, N_TOK], dt.float32, tag="o")
                            for jc in range(2):
                                jsz = 128 if jc == 0 else N_TOK - 128
                                nc.tensor.matmul(
                                    o_ps[:],
                                    lhsT=V[b][:jsz, jc, h * HW:(h + 1) * HW],
                                    rhs=eT[:jsz, jc, :],
                                    start=(jc == 0),
                                    stop=(jc == 1),
                                )
                            # fast evict (frees the PSUM slot); normalization
                            # is a deferred batched pass below
                            u = U[(b % 2) * HEADS + h]
                            if parity == 0:
                                nc.scalar.copy(out=u[:], in_=o_ps[:])
                            else:
                                nc.vector.tensor_copy(out=u[:], in_=o_ps[:])

                # ---- deferred normalization for this chunk ------------
                for b in (2 * nt, 2 * nt + 1):
                    i0 = (b % 2) * N_TOK
                    for h in range(HEADS):
                        po = 64 * (h % 2)
                        oh = h // 2
                        u = U[(b % 2) * HEADS + h]
                        r0 = pR.tile([1, N_TOK], dt.float32, tag="r0")
                        nc.vector.tensor_copy(out=r0[:], in_=u[D:HW, :])
                        r1 = pR.tile([1, N_TOK], dt.float32, tag="r1")
                        nc.vector.reciprocal_approx_fast(out=r1[:], in_=r0[:])
                        rb = pR.tile([64, N_TOK], dt.float32, tag="rb")
                        nc.gpsimd.partition_broadcast(rb[:], r1[:])
                        nc.vector.tensor_mul(
                            OTt[nt][po:po + 64, oh, i0:i0 + N_TOK],
                            u[0:D, :],
                            rb[:],
                        )

                # ---- projection for the previous t-chunk --------------
                for pnt in ([nt - 1] if nt > 0 else []) + (
                    [nt] if nt == N_NT - 1 else []
                ):
                    psl = slice(pnt * NT, (pnt + 1) * NT)
                    for ct in range(OT):
                        ps0 = pA.tile([128, 512], dt.float32, tag="ps2k", name="psd")
                        ps = ps0[:, :NT]
                        for oc in range(OT):
                            nc.tensor.matmul(
                                ps[:],
                                lhsT=pw[:, oc, ct * 128:(ct + 1) * 128],
                                rhs=OTt[pnt][:, oc, :],
                                start=(oc == 0),
                                stop=(oc == OT - 1),
                            )
                        st = pE.tile([128, NT], dt.float32, tag="st")
                        nc.scalar.activation(
                            out=st[:], in_=ps[:], func=AF.Identity,
                            bias=bp[:, ct:ct + 1], scale=1.0,
                        )
                        nc.sync.dma_start(
                            out=out_d[ct * 128:(ct + 1) * 128, psl], in_=st[:]
                        )

    nc.compile()
    return nc


def _prep_inputs(x, qkv_w, qkv_b, proj_w, proj_b, down_w, down_b, up_w, up_b):
    f32 = np.float32
    x = np.asarray(x, f32)
    qkv_w = np.asarray(qkv_w, f32)
    qkv_b = np.asarray(qkv_b, f32)
    proj_w = np.asarray(proj_w, f32)
    proj_b = np.asarray(proj_b, f32)
    down_w = np.asarray(down_w, f32)
    down_b = np.asarray(down_b, f32)
    up_w = np.asarray(up_w, f32)
    up_b = np.asarray(up_b, f32)

    wq = qkv_w[0:DIM]
    wk = qkv_w[DIM:2 * DIM]
    wv = qkv_w[2 * DIM:3 * DIM]
    bq = qkv_b[0:DIM]
    bk = qkv_b[DIM:2 * DIM] + P_SCALE * up_b[0:DIM]
    bv = qkv_b[2 * DIM:3 * DIM] + P_SCALE * up_b[DIM:2 * DIM]
    # v-bias rides through the softmax average unchanged -> fold into proj_b
    bp = proj_b + proj_w @ bv

    def t_bf16(a):
        return np.ascontiguousarray(a.T).astype(BF16)

    def b_lay(vec, nt):
        return np.ascontiguousarray(vec.reshape(nt, 128).T).astype(f32)

    common = {
        "wq": t_bf16(wq),
        "wk": t_bf16(wk),
        "wv": t_bf16(wv),
        "dw": t_bf16(down_w),
        "uk": t_bf16(P_SCALE * up_w[0:DIM]),
        "uv": t_bf16(P_SCALE * up_w[DIM:2 * DIM]),
        "pw": t_bf16(proj_w),
        "bq": b_lay(bq, OT),
        "bk": b_lay(bk, OT),
        "bd": b_lay(down_b, MC),
        "bp": b_lay(bp, OT),
    }
    in_maps = []
    for c in range(N_CORES):
        xc = x[c * B_LOC:(c + 1) * B_LOC].reshape(T, DIM)
        m = dict(common)
        m["xT"] = np.ascontiguousarray(xc.T).astype(BF16)
        in_maps.append(m)
    return in_maps


def kernel(x, qkv_w, qkv_b, proj_w, proj_b, down_w, down_b, up_w, up_b):
    from concourse.bass_utils import run_bass_kernel_spmd

    if "nc" not in _CACHE:
        _CACHE["nc"] = _build()
    nc = _CACHE["nc"]

    in_maps = _prep_inputs(
        x, qkv_w, qkv_b, proj_w, proj_b, down_w, down_b, up_w, up_b
    )
    res = run_bass_kernel_spmd(nc, in_maps, list(range(N_CORES)))
    outs = []
    for i in range(N_CORES):
        oT = np.asarray(res.results[i]["outT"], dtype=np.float32)
        outs.append(np.ascontiguousarray(oT.T).reshape(B_LOC, N_TOK, DIM))
    return np.concatenate(outs, axis=0)


# revision 25
# speedup vs baseline: 1.2024x; 1.1473x over previous
"""AdapterAttention Trainium2 kernel (8 NeuronCores, batch-data-parallel).

Reference computation (per image, N=197 tokens, C=768, 12 heads x 64):
    mid       = tanh(x @ down_w.T + down_b)                  [N, 256]
    kv_prefix = mid @ up_w.T + up_b                          [N, 1536]
    qkv       = x @ qkv_w.T + qkv_b + 0.8*[0, kv_prefix]     [N, 2304]
    attn      = softmax(q k^T / 8) v  per head
    out       = attn_out @ proj_w.T + proj_b

Distribution: batch 64 -> 8 images per core, weights replicated. No
collectives; host shards inputs and reassembles outputs.

On-chip layout strategy (everything "feature-major"/transposed so no
on-chip transposes are needed; host pre-transposes x and weights):
    xT    [c=768, t=1576]  (t = 8 images * 197 tokens)
    qT/kT [o=768, t]  = W^T-matmuls on xT (+ adapter matmuls for kT)
    V_aug per image, natural [tok chunks, 12*(64+1)]; the extra column
          per head is constant 1.0, so the AV matmul's PSUM row 64
          yields the softmax denominator for free.
    S^T   [j, i] per (image, head) = k-slice^T-matmul(q) -> one exp op
          (bf16 out IS the E^T eviction, no separate copy)
    O^T   [65, i] = V_aug-lhsT matmul(E^T); normalization = reciprocal
          of row 64, gpsimd partition-broadcast, fused into the output
          eviction as a tensor_tensor multiply.
    proj  -> outT [co, t]; host transposes back.
The v-bias is folded into proj bias on the host (softmax rows sum to 1);
0.8 prefix scale is folded into up_w/up_b on the host.
"""

import sys

sys.path.insert(0, "/opt/trn_rl_repo")

import ml_dtypes
import numpy as np

DIM = 768
MID = 256
HEADS = 12
D = 64
P_SCALE = 0.8
SCALE = D ** -0.5
B_FULL = 64
N_TOK = 197
N_CORES = 8
B_LOC = B_FULL // N_CORES          # 8 images per core
T = B_LOC * N_TOK                  # 1576 tokens per core
NT = 394                           # t-chunk for dense matmuls (= 2 images)
N_NT = T // NT                     # 4
CC = DIM // 128                    # 6 contraction chunks over c
MC = MID // 128                    # 2 contraction chunks over mid
OT = DIM // 128                    # 6 output tiles over o / co
HW = D + 1                         # 65: per-head V columns incl. ones col

BF16 = ml_dtypes.bfloat16

_CACHE = {}
SAFE_EXP = False


def _build():
    from contextlib import ExitStack

    import concourse.tile as tile
    from concourse import bacc, mybir

    dt = mybir.dt
    AF = mybir.ActivationFunctionType

    nc = bacc.Bacc(
        "TRN2", target_bir_lowering=False, debug=False, num_devices=N_CORES
    )

    def din(name, shape, dtype):
        return nc.dram_tensor(name, shape, dtype, kind="ExternalInput").ap()

    xT_d = din("xT", [DIM, T], dt.bfloat16)
    wq_d = din("wq", [DIM, DIM], dt.bfloat16)      # [c, o]
    wk_d = din("wk", [DIM, DIM], dt.bfloat16)
    wv_d = din("wv", [DIM, DIM], dt.bfloat16)
    dw_d = din("dw", [DIM, MID], dt.bfloat16)      # down_w.T  [c, m]
    uk_d = din("uk", [MID, DIM], dt.bfloat16)      # 0.8*up_w[:C].T  [m, o]
    uv_d = din("uv", [MID, DIM], dt.bfloat16)      # 0.8*up_w[C:].T  [m, o]
    pw_d = din("pw", [DIM, DIM], dt.bfloat16)      # proj_w.T  [o, co]
    bq_d = din("bq", [128, OT], dt.float32)
    bk_d = din("bk", [128, OT], dt.float32)
    bd_d = din("bd", [128, MC], dt.float32)
    bp_d = din("bp", [128, OT], dt.float32)
    out_d = nc.dram_tensor("outT", [DIM, T], dt.float32, kind="ExternalOutput").ap()

    with tile.TileContext(nc) as tc, ExitStack() as ctx:
        persist = ctx.enter_context(tc.tile_pool(name="persist", bufs=1))

        xT = persist.tile([128, CC, T], dt.bfloat16)
        xT_v = xT_d.rearrange("(c p) t -> p c t", p=128)
        nc.sync.dma_start(out=xT[:, :, 0:NT], in_=xT_v[:, :, 0:NT])
        dw = persist.tile([128, CC, MID], dt.bfloat16)
        nc.sync.dma_start(out=dw[:], in_=dw_d.rearrange("(c p) m -> p c m", p=128))
        wq = persist.tile([128, CC, DIM], dt.bfloat16)
        nc.sync.dma_start(out=wq[:], in_=wq_d.rearrange("(c p) o -> p c o", p=128))
        # later-needed weights are DMA'd from the Scalar engine's stream at
        # points inside stage A(nt=0), so their transfers don't steal HBM
        # bandwidth from dw/xT/wq which gate the first matmuls
        wk = persist.tile([128, CC, DIM], dt.bfloat16)
        uk = persist.tile([128, MC, DIM], dt.bfloat16)
        wv = persist.tile([128, CC, DIM], dt.bfloat16)
        uv = persist.tile([128, MC, DIM], dt.bfloat16)
        pw = persist.tile([128, OT, DIM], dt.bfloat16)
        bq = persist.tile([128, OT], dt.float32)
        nc.sync.dma_start(out=bq[:], in_=bq_d)
        bk = persist.tile([128, OT], dt.float32)
        nc.sync.dma_start(out=bk[:], in_=bk_d)
        bd = persist.tile([128, MC], dt.float32)
        nc.sync.dma_start(out=bd[:], in_=bd_d)
        bp = persist.tile([128, OT], dt.float32)
        nc.sync.dma_start(out=bp[:], in_=bp_d)
        for _nt in range(1, N_NT):
            _sl = slice(_nt * NT, (_nt + 1) * NT)
            nc.sync.dma_start(out=xT[:, :, _sl], in_=xT_v[:, :, _sl])

        # per-nt-chunk activation tiles so attention can overlap stage A
        qT = [persist.tile([128, OT, NT], dt.bfloat16, name=f"qT{i}") for i in range(N_NT)]
        kT = [persist.tile([128, OT, NT], dt.bfloat16, name=f"kT{i}") for i in range(N_NT)]
        OTt = [persist.tile([128, OT, NT], dt.bfloat16, name=f"OTt{i}") for i in range(N_NT)]
        midT = [persist.tile([128, MC, NT], dt.bfloat16, name=f"midT{i}") for i in range(N_NT)]
        # V augmented with a ones column per head: [tok, 12*(64+1)]
        V = [
            persist.tile([128, 2, HEADS * HW], dt.bfloat16, name=f"V{b}")
            for b in range(B_LOC)
        ]
        for b in range(B_LOC):
            ones_cols = V[b][:].rearrange("p j (h c) -> p j h c", h=HEADS)[
                :, :, :, D:HW
            ]
            nc.vector.memset(ones_cols, 1.0)

        uPool = ctx.enter_context(tc.tile_pool(name="unorm", bufs=2))
        U = [
            uPool.tile([HW, N_TOK], dt.float32, tag=f"u{i}", name=f"u{i}")
            for i in range(2 * HEADS)
        ]
        with (
            tc.tile_pool(name="psAB", bufs=2, space="PSUM") as pA,
            tc.tile_pool(name="psS", bufs=2, space="PSUM") as pS,
            tc.tile_pool(name="psO", bufs=2, space="PSUM") as pO,
            tc.tile_pool(name="psD", bufs=2, space="PSUM") as pD,
            tc.tile_pool(name="att", bufs=3) as pE,
            tc.tile_pool(name="rec", bufs=4) as pR,
        ):
            # ---- Stage A/B per t-chunk: midT, qT, kT, V ---------------
            for nt in range(N_NT):
                sl = slice(nt * NT, (nt + 1) * NT)
                for mt in range(MC):
                    ps0 = pA.tile([128, 512], dt.float32, tag="ps2k", name="psm")
                    ps = ps0[:, :NT]
                    for cc in range(CC):
                        nc.tensor.matmul(
                            ps[:],
                            lhsT=dw[:, cc, mt * 128:(mt + 1) * 128],
                            rhs=xT[:, cc, sl],
                            start=(cc == 0),
                            stop=(cc == CC - 1),
                        )
                    nc.scalar.activation(
                        out=midT[nt][:, mt, :], in_=ps[:], func=AF.Tanh,
                        bias=bd[:, mt:mt + 1], scale=1.0,
                    )
                if nt == 0:
                    nc.scalar.dma_start(
                        out=wk[:], in_=wk_d.rearrange("(c p) o -> p c o", p=128)
                    )
                    nc.scalar.dma_start(
                        out=uk[:], in_=uk_d.rearrange("(c p) o -> p c o", p=128)
                    )
                for ot in range(OT):
                    ps0 = pA.tile([128, 512], dt.float32, tag="ps2k", name="psq")
                    ps = ps0[:, :NT]
                    for cc in range(CC):
                        nc.tensor.matmul(
                            ps[:],
                            lhsT=wq[:, cc, ot * 128:(ot + 1) * 128],
                            rhs=xT[:, cc, sl],
                            start=(cc == 0),
                            stop=(cc == CC - 1),
                        )
                    nc.scalar.activation(
                        out=qT[nt][:, ot, :], in_=ps[:], func=AF.Identity,
                        bias=bq[:, ot:ot + 1], scale=1.0,
                    )
                    if nt == 0 and ot == 1:
                        nc.scalar.dma_start(
                            out=wv[:],
                            in_=wv_d.rearrange("(c p) o -> p c o", p=128),
                        )
                        nc.scalar.dma_start(
                            out=uv[:],
                            in_=uv_d.rearrange("(c p) o -> p c o", p=128),
                        )
                    if nt == 0 and ot == 3:
                        nc.scalar.dma_start(
                            out=pw[:],
                            in_=pw_d.rearrange("(c p) o -> p c o", p=128),
                        )
                for ot in range(OT):
                    ps0 = pA.tile([128, 512], dt.float32, tag="ps2k", name="psk")
                    ps = ps0[:, :NT]
                    for cc in range(CC):
                        nc.tensor.matmul(
                            ps[:],
                            lhsT=wk[:, cc, ot * 128:(ot + 1) * 128],
                            rhs=xT[:, cc, sl],
                            start=(cc == 0),
                            stop=False,
                        )
                    for mc in range(MC):
                        nc.tensor.matmul(
                            ps[:],
                            lhsT=uk[:, mc, ot * 128:(ot + 1) * 128],
                            rhs=midT[nt][:, mc, :],
                            start=False,
                            stop=(mc == MC - 1),
                        )
                    nc.scalar.activation(
                        out=kT[nt][:, ot, :], in_=ps[:], func=AF.Identity,
                        bias=bk[:, ot:ot + 1], scale=1.0,
                    )
                # V for the two images inside this t-chunk
                for b in (2 * nt, 2 * nt + 1):
                    t0 = b * N_TOK
                    l0 = (b % 2) * N_TOK     # image offset inside nt tiles
                    for jc in range(2):
                        jsz = 128 if jc == 0 else N_TOK - 128
                        tok = slice(t0 + jc * 128, t0 + jc * 128 + jsz)
                        ltok = slice(l0 + jc * 128, l0 + jc * 128 + jsz)
                        for osl, nh, h0 in (
                            (slice(0, 512), 8, 0),
                            (slice(512, DIM), 4, 8),
                        ):
                            ow = osl.stop - osl.start
                            ps = pA.tile([128, 512], dt.float32, tag="ps2k", name="psv")
                            for cc in range(CC):
                                nc.tensor.matmul(
                                    ps[:jsz, :ow],
                                    lhsT=xT[:, cc, tok],
                                    rhs=wv[:, cc, osl],
                                    start=(cc == 0),
                                    stop=False,
                                )
                            for mc in range(MC):
                                nc.tensor.matmul(
                                    ps[:jsz, :ow],
                                    lhsT=midT[nt][:, mc, ltok],
                                    rhs=uv[:, mc, osl],
                                    start=False,
                                    stop=(mc == MC - 1),
                                )
                            dst = V[b][:].rearrange(
                                "p j (h c) -> p j h c", h=HEADS
                            )[:jsz, jc, h0:h0 + nh, 0:D]
                            src = ps[:jsz, :ow].rearrange(
                                "p (h c) -> p h c", h=nh
                            )
                            if b % 2 == 0:
                                nc.vector.tensor_copy(out=dst, in_=src)
                            else:
                                nc.scalar.copy(out=dst, in_=src)

                # ---- attention for the two images of this chunk -------
                for b in (2 * nt, 2 * nt + 1):
                    i0 = (b % 2) * N_TOK     # image offset inside nt tiles
                    for h in range(HEADS):
                        po = 64 * (h % 2)
                        oh = h // 2
                        s_ps = pS.tile([128, 2, N_TOK], dt.float32, tag="s")
                        for jc in range(2):
                            jsz = 128 if jc == 0 else N_TOK - 128
                            jsl = slice(i0 + jc * 128, i0 + jc * 128 + jsz)
                            nc.tensor.matmul(
                                s_ps[:jsz, jc, :],
                                lhsT=kT[nt][po:po + 64, oh, jsl],
                                rhs=qT[nt][po:po + 64, oh, i0:i0 + N_TOK],
                                start=True,
                                stop=True,
                            )
                        if True:
                            parity = h % 2
                            eT = pE.tile([128, 2, N_TOK], dt.bfloat16, tag="eT")
                            # one exp over both j-chunks; rows 69.. of chunk
                            # 1 are stale psum, never read downstream
                            nc.scalar.activation(
                                out=eT[:], in_=s_ps[:], func=AF.Exp, scale=SCALE
                            )
                            o_ps = pO.tile([HW, N_TOK], dt.float32, tag="o")
                            for jc in range(2):
                                jsz = 128 if jc == 0 else N_TOK - 128
                                nc.tensor.matmul(
                                    o_ps[:],
                                    lhsT=V[b][:jsz, jc, h * HW:(h + 1) * HW],
                                    rhs=eT[:jsz, jc, :],
                                    start=(jc == 0),
                                    stop=(jc == 1),
                                )
                            # fast evict (frees the PSUM slot); normalization
                            # is a deferred batched pass below
                            u = U[(b % 2) * HEADS + h]
                            if parity == 0:
                                nc.scalar.copy(out=u[:], in_=o_ps[:])
                            else:
                                nc.vector.tensor_copy(out=u[:], in_=o_ps[:])

                # ---- deferred normalization for this chunk ------------
                for b in (2 * nt, 2 * nt + 1):
                    i0 = (b % 2) * N_TOK
                    for h in range(HEADS):
                        po = 64 * (h % 2)
                        oh = h // 2
                        u = U[(b % 2) * HEADS + h]
                        r0 = pR.tile([1, N_TOK], dt.float32, tag="r0")
                        nc.vector.tensor_copy(out=r0[:], in_=u[D:HW, :])
                        r1 = pR.tile([1, N_TOK], dt.float32, tag="r1")
                        nc.vector.reciprocal_approx_fast(out=r1[:], in_=r0[:])
                        rb = pR.tile([64, N_TOK], dt.float32, tag="rb")
                        nc.gpsimd.partition_broadcast(rb[:], r1[:])
                        nc.vector.tensor_mul(
                            OTt[nt][po:po + 64, oh, i0:i0 + N_TOK],
                            u[0:D, :],
                            rb[:],
                        )

                # ---- projection for the previous t-chunk --------------
                for pnt in ([nt - 1] if nt > 0 else []) + (
                    [nt] if nt == N_NT - 1 else []
                ):
                    psl = slice(pnt * NT, (pnt + 1) * NT)
                    for ct in range(OT):
                        ps = pD.tile([128, NT], dt.float32, tag="psd", name="psd")
                        for oc in range(OT):
                            nc.tensor.matmul(
                                ps[:],
                                lhsT=pw[:, oc, ct * 128:(ct + 1) * 128],
                                rhs=OTt[pnt][:, oc, :],
                                start=(oc == 0),
                                stop=(oc == OT - 1),
                            )
                        st = pE.tile([128, NT], dt.float32, tag="st")
                        nc.scalar.activation(
                            out=st[:], in_=ps[:], func=AF.Identity,
                            bias=bp[:, ct:ct + 1], scale=1.0,
                        )
                        nc.sync.dma_start(
                            out=out_d[ct * 128:(ct + 1) * 128, psl], in_=st[:]
                        )

    nc.compile()
    return nc


def _prep_inputs(x, qkv_w, qkv_b, proj_w, proj_b, down_w, down_b, up_w, up_b):
    f32 = np.float32
    x = np.asarray(x, f32)
    qkv_w = np.asarray(qkv_w, f32)
    qkv_b = np.asarray(qkv_b, f32)
    proj_w = np.asarray(proj_w, f32)
    proj_b = np.asarray(proj_b, f32)
    down_w = np.asarray(down_w, f32)
    down_b = np.asarray(down_b, f32)
    up_w = np.asarray(up_w, f32)
    up_b = np.asarray(up_b, f32)

    wq = qkv_w[0:DIM]
    wk = qkv_w[DIM:2 * DIM]
    wv = qkv_w[2 * DIM:3 * DIM]
    bq = qkv_b[0:DIM]
    bk = qkv_b[DIM:2 * DIM] + P_SCALE * up_b[0:DIM]
    bv = qkv_b[2 * DIM:3 * DIM] + P_SCALE * up_b[DIM:2 * DIM]
    # v-bias rides through the softmax average unchanged -> fold into proj_b
    bp = proj_b + proj_w @ bv

    def t_bf16(a):
        return np.ascontiguousarray(a.T).astype(BF16)

    def b_lay(vec, nt):
        return np.ascontiguousarray(vec.reshape(nt, 128).T).astype(f32)

    common = {
        "wq": t_bf16(wq),
        "wk": t_bf16(wk),
        "wv": t_bf16(wv),
        "dw": t_bf16(down_w),
        "uk": t_bf16(P_SCALE * up_w[0:DIM]),
        "uv": t_bf16(P_SCALE * up_w[DIM:2 * DIM]),
        "pw": t_bf16(proj_w),
        "bq": b_lay(bq, OT),
        "bk": b_lay(bk, OT),
        "bd": b_lay(down_b, MC),
        "bp": b_lay(bp, OT),
    }
    in_maps = []
    for c in range(N_CORES):
        xc = x[c * B_LOC:(c + 1) * B_LOC].reshape(T, DIM)
        m = dict(common)
        m["xT"] = np.ascontiguousarray(xc.T).astype(BF16)
        in_maps.append(m)
    return in_maps


def kernel(x, qkv_w, qkv_b, proj_w, proj_b, down_w, down_b, up_w, up_b):
    from concourse.bass_utils import run_bass_kernel_spmd

    if "nc" not in _CACHE:
        _CACHE["nc"] = _build()
    nc = _CACHE["nc"]

    in_maps = _prep_inputs(
        x, qkv_w, qkv_b, proj_w, proj_b, down_w, down_b, up_w, up_b
    )
    res = run_bass_kernel_spmd(nc, in_maps, list(range(N_CORES)))
    outs = []
    for i in range(N_CORES):
        oT = np.asarray(res.results[i]["outT"], dtype=np.float32)
        outs.append(np.ascontiguousarray(oT.T).reshape(B_LOC, N_TOK, DIM))
    return np.concatenate(outs, axis=0)
